# revision 1
# baseline (speedup 1.0000x reference)
"""Trainium2 Bass kernel for nn_Autoencoder_65223373357102 (FLAME-style autoencoder).

Strategy:
  Phase 1 (8-way tensor parallel): encoder GEMM [64,150528]@[150528,556] sharded
  along the input-feature axis. Each core transposes its x shard on TensorE,
  multiplies against its 1/8 slice of enc_W, adds enc_b/8 via a K=1 matmul, and
  AllReduces the [64,556] latent (142 KB).
  Phase 2 (replicated): blendshape GEMM [64,400]@[400,3*5023] in plane-separated
  layout + all per-batch geometry with batch on partitions; per-batch scalars are
  broadcast along the free axis via tensor_scalar. Every core computes the full
  output; the host takes core 0's copy.
"""
import sys
import types

sys.path.insert(0, "/opt/trn_rl_repo")

import numpy as np


def _ensure_ntff_hook():
    """Provide antenv.axon_hooks + install the ctypes NTFF profile hook so
    run_bass_kernel_spmd(trace=True) can pull a neuron-profile under axon."""
    name = "antenv.axon_hooks"
    if name not in sys.modules:
        mod = types.ModuleType(name)
        mod._HOOK = None

        def set_axon_ntff_profile_hook(hook):
            mod._HOOK = hook

        def get_axon_ntff_profile_hook():
            return mod._HOOK

        mod.set_axon_ntff_profile_hook = set_axon_ntff_profile_hook
        mod.get_axon_ntff_profile_hook = get_axon_ntff_profile_hook
        sys.modules[name] = mod
        try:
            import antenv

            antenv.axon_hooks = mod
        except ImportError:
            pass
    mod = sys.modules[name]
    if mod.get_axon_ntff_profile_hook() is None:
        try:
            from trn_agent_boot.trn_boot import _ntff_profile_via_ctypes

            hook = _ntff_profile_via_ctypes("/opt/axon/libaxon_pjrt.so")
            if hook is not None:
                mod.set_axon_ntff_profile_hook(hook)
        except Exception:
            pass


_ensure_ntff_hook()

from concourse import bass, mybir, tile
from concourse.bass_utils import run_bass_kernel_spmd

F32 = mybir.dt.float32
ALU = mybir.AluOpType
ACTF = mybir.ActivationFunctionType
AX = mybir.AxisListType

B = 64
V = 5023
VM = 3500
LAT = 556
DIN = 3 * 224 * 224  # 150528
NCORES = 8
KSH = DIN // NCORES  # 18816
KTILES = KSH // 128  # 147
NOUT = 2 * VM + 68 + 11  # 7079
GAZE_DIR = -1.0
HALF_PI = 1.5707963267948966


def _chunks(total, step):
    out = []
    o = 0
    while o < total:
        out.append((o, min(step, total - o)))
        o += step
    return out


class Geo:
    """Helper for tiny per-batch scalar ops on [rows,1] tiles."""

    _uid = [0]

    def __init__(self, nc, pool, rows=B):
        self.nc = nc
        self.pool = pool
        self.rows = rows

    def t(self, cols=1):
        Geo._uid[0] += 1
        return self.pool.tile([self.rows, cols], F32, name=f"g{Geo._uid[0]}_{cols}")

    def mul(self, a, b):
        o = self.t()
        self.nc.vector.tensor_tensor(out=o, in0=a, in1=b, op=ALU.mult)
        return o

    def add(self, a, b):
        o = self.t()
        self.nc.vector.tensor_tensor(out=o, in0=a, in1=b, op=ALU.add)
        return o

    def sub(self, a, b):
        o = self.t()
        self.nc.vector.tensor_tensor(out=o, in0=a, in1=b, op=ALU.subtract)
        return o

    def mac(self, a, s, acc):
        """(a * s) + acc, s is a [B,1] AP scalar."""
        o = self.t()
        self.nc.vector.scalar_tensor_tensor(
            out=o, in0=a, scalar=s, in1=acc, op0=ALU.mult, op1=ALU.add
        )
        return o

    def dot3(self, ax, ay, az, bx, by, bz):
        o = self.mul(ax, bx)
        o = self.mac(ay, by, o)
        o = self.mac(az, bz, o)
        return o

    def cross3(self, ax, ay, az, bx, by, bz):
        """a x b -> 3 [B,1] tiles."""
        cx = self.sub(self.mul(ay, bz), self.mul(az, by))
        cy = self.sub(self.mul(az, bx), self.mul(ax, bz))
        cz = self.sub(self.mul(ax, by), self.mul(ay, bx))
        return cx, cy, cz


def axis_angle_R(nc, g, aa3, pfx, halfpi):
    R_ = g.rows
    """aa3: [B,3] axis-angle tile -> R [B,9] tile, R[l,i] at col l*3+i.

    R = c*I + s*K + (1-c) a a^T  (Rodrigues, matching reference)
    """
    pool = g.pool
    sq = pool.tile([R_, 3], F32, name=pfx + "aaR_sq")
    nc.vector.tensor_tensor(out=sq, in0=aa3, in1=aa3, op=ALU.mult)
    th2 = g.t()
    nc.vector.tensor_reduce(out=th2, in_=sq, axis=AX.X, op=ALU.add)
    theta = g.t()
    nc.scalar.activation(out=theta, in_=th2, func=ACTF.Sqrt)
    thm = g.t()
    nc.vector.tensor_scalar_max(out=thm, in0=theta, scalar1=1e-8)
    rth = g.t()
    nc.vector.reciprocal(out=rth, in_=thm)
    axis3 = pool.tile([R_, 3], F32, name=pfx + "aaR_axis")
    nc.vector.tensor_scalar_mul(out=axis3, in0=aa3, scalar1=rth)
    s = g.t()
    nc.scalar.activation(out=s, in_=theta, func=ACTF.Sin)
    c = g.t()
    nc.scalar.activation(out=c, in_=theta, func=ACTF.Sin, bias=halfpi)
    omc = g.t()
    nc.vector.tensor_scalar(
        out=omc, in0=c, scalar1=-1.0, scalar2=1.0, op0=ALU.mult, op1=ALU.add
    )
    ax, ay, az = axis3[:, 0:1], axis3[:, 1:2], axis3[:, 2:3]
    # diag: omc*a_i^2 + c
    asq = pool.tile([R_, 3], F32, name=pfx + "aaR_asq")
    nc.vector.tensor_tensor(out=asq, in0=axis3, in1=axis3, op=ALU.mult)
    R = pool.tile([R_, 9], F32, name=pfx + "aaR_R")
    dmul = pool.tile([R_, 3], F32, name=pfx + "aaR_dmul")
    nc.vector.tensor_scalar_mul(out=dmul, in0=asq, scalar1=omc)
    # s*a
    sa = pool.tile([R_, 3], F32, name=pfx + "aaR_sa")
    nc.vector.tensor_scalar_mul(out=sa, in0=axis3, scalar1=s)
    sax, say, saz = sa[:, 0:1], sa[:, 1:2], sa[:, 2:3]
    # off-diag products omc*ai*aj
    mxy = g.mul(g.mul(ax, ay), omc)
    mxz = g.mul(g.mul(ax, az), omc)
    myz = g.mul(g.mul(ay, az), omc)
    # assemble diag: R[l*4] = dmul_l + c
    for l in range(3):
        nc.vector.tensor_tensor(
            out=R[:, 4 * l:4 * l + 1], in0=dmul[:, l:l + 1], in1=c, op=ALU.add
        )
    nc.vector.tensor_tensor(out=R[:, 1:2], in0=mxy, in1=saz, op=ALU.subtract)  # R01
    nc.vector.tensor_tensor(out=R[:, 2:3], in0=mxz, in1=say, op=ALU.add)  # R02
    nc.vector.tensor_tensor(out=R[:, 3:4], in0=mxy, in1=saz, op=ALU.add)  # R10
    nc.vector.tensor_tensor(out=R[:, 5:6], in0=myz, in1=sax, op=ALU.subtract)  # R12
    nc.vector.tensor_tensor(out=R[:, 6:7], in0=mxz, in1=say, op=ALU.subtract)  # R20
    nc.vector.tensor_tensor(out=R[:, 7:8], in0=myz, in1=sax, op=ALU.add)  # R21
    return R


_ENG_ATTR = {
    "SP": "sync", "Pool": "gpsimd", "PE": "tensor",
    "DVE": "vector", "Activation": "scalar",
}


def _legalize_waits(nc):
    """This walrus accepts only one sync-wait slot per instruction; move extra
    waits onto same-engine NoOps inserted right before the instruction."""
    import concourse.mybir as _mybir

    def make_nop(engine):
        eng = getattr(nc, _ENG_ATTR[engine.name])
        bi = eng.nop(nofuse=True)
        mi = bi.ins
        for bb in nc.main_func.blocks:
            if bb.instructions and bb.instructions[-1].name == mi.name:
                bb.instructions.pop()
                break
        mi.engine = engine
        return mi

    for bb in nc.main_func.blocks:
        snapshot = list(bb.instructions)
        newlist = []
        changed = False
        for inst in snapshot:
            si = inst.sync_info
            waits = list(si.on_wait) if (si and si.on_wait) else []
            if (
                len(waits) > 1
                and not inst.name.startswith("barrier")
                and inst.engine is not None
                and getattr(inst.engine, "name", None) in _ENG_ATTR
            ):
                for w in waits[:-1]:
                    nop = make_nop(inst.engine)
                    nop.sync_info = _mybir.SyncInfo(on_wait=[w], on_update=[])
                    newlist.append(nop)
                inst.sync_info = _mybir.SyncInfo(
                    on_wait=[waits[-1]], on_update=list(si.on_update)
                )
                changed = True
            newlist.append(inst)
        if changed:
            bb.instructions[:] = newlist


def build_graph(fl_idx, idx4, idx2, l_lo, r_lo):
    """fl_idx: 68 ints (vert cols for masked landmarks), idx4/idx2: landmark vert
    cols, l_lo/r_lo: start of the contiguous eye ranges."""
    nc = bass.Bass(target_bir_lowering=False)

    x_p = nc.declare_dram_parameter("x_sh", [KSH, B], F32, isOutput=False)
    w_p = nc.declare_dram_parameter("w_sh", [KSH, LAT], F32, isOutput=False)
    b_p = nc.declare_dram_parameter("enc_b", [1, LAT + 128 + 3], F32, isOutput=False)
    bm_p = nc.declare_dram_parameter("bmean", [128, 12], F32, isOutput=False)
    tpl_p = nc.declare_dram_parameter("tmpl", [3, V], F32, isOutput=False)
    bas_p = nc.declare_dram_parameter("basis", [400, 3, V], F32, isOutput=False)
    cam_p = nc.declare_dram_parameter("cam", [B, 12], F32, isOutput=False)
    out_p = nc.declare_dram_parameter("out", [B, 3, NOUT], F32, isOutput=True)

    ar_in = nc.dram_tensor("ar_in", [B, LAT], F32)
    ar_out = nc.dram_tensor("ar_out", [B, LAT], F32, addr_space="Shared")

    with tile.TileContext(nc) as tc:
        with (
            tc.tile_pool(name="consts", bufs=1) as consts,
            tc.tile_pool(name="latents", bufs=1) as latp,
            tc.tile_pool(name="geo", bufs=1) as geop,
            tc.tile_pool(name="planes", bufs=1) as planep,
            tc.tile_pool(name="dum", bufs=1, space="PSUM") as dum,
        ):
            b_sb = consts.tile([1, LAT + 128 + 3], F32)
            nc.sync.dma_start(out=b_sb, in_=b_p[:, :])
            ones8 = b_sb[:, LAT:LAT + B]       # value 1/NCORES, packed by host
            ones1 = b_sb[:, LAT + B:LAT + 2 * B]  # value 1.0, packed by host
            halfpi = consts.tile([128, 1], F32)
            nc.vector.memset(halfpi, HALF_PI)
            # PE matmuls carry a single sync-wait slot on this walrus; dummy
            # 1-wait matmuls make PE observe one dep before the real matmul.
            d1 = dum.tile([1, 1], F32)
            d64 = dum.tile([B, 1], F32)

            # ---------------- Phase 1: encoder GEMM ----------------
            NSPL = [(0, 512), (512, 44)]
            TPC = 7  # k-tiles per x chunk
            with (
                tc.tile_pool(name="xin", bufs=3) as xin,
                tc.tile_pool(name="wts", bufs=3) as wts,
                tc.tile_pool(name="encp", bufs=1, space="PSUM") as encp,
            ):
                pe = [encp.tile([B, n], F32, name=f"pe{j}", tag=f"pe{j}") for j, (_, n) in enumerate(NSPL)]
                x_view = x_p.ap().rearrange("(c t p) m -> c p t m", t=TPC, p=128)
                w_view = w_p.ap().rearrange("(c t p) m -> c p t m", t=TPC, p=128)
                for ci in range(KTILES // TPC):
                    x_c = xin.tile([128, TPC, B], F32)
                    nc.gpsimd.dma_start(out=x_c, in_=x_view[ci])
                    nc.tensor.matmul(
                        d1, lhsT=x_c[:, 0, 0:1], rhs=x_c[:, 0, 0:1],
                        start=True, stop=True, skip_group_check=True,
                    )
                    w_c = wts.tile([128, TPC, LAT], F32)
                    nc.sync.dma_start(out=w_c, in_=w_view[ci])
                    for t in range(TPC):
                        k = ci * TPC + t
                        for j, (n0, n) in enumerate(NSPL):
                            nc.tensor.matmul(
                                pe[j],
                                lhsT=x_c[:, t, :],
                                rhs=w_c[:, t, n0:n0 + n],
                                start=(k == 0),
                                stop=False,
                            )
                for j, (n0, n) in enumerate(NSPL):
                    nc.tensor.matmul(
                        pe[j],
                        lhsT=ones8,
                        rhs=b_sb[:, n0:n0 + n],
                        start=False,
                        stop=True,
                    )
                lat1 = latp.tile([B, LAT], F32)
                for j, (n0, n) in enumerate(NSPL):
                    nc.vector.tensor_copy(out=lat1[:, n0:n0 + n], in_=pe[j])
                nc.sync.dma_start(out=ar_in[:, :], in_=lat1)

            # prefetch the first basis chunks before the collective so the
            # DMA engines stay busy through the AllReduce bubble
            basp_ctx = tc.tile_pool(name="bas", bufs=12)
            basp = basp_ctx.__enter__()
            KSPL = [(0, 128, 128), (128, 128, 128), (256, 128, 128), (384, 16, 32)]
            VCH = _chunks(V, 512)
            bts = {}
            for j in (0, 1, 2):
                n0, n = VCH[j]
                for ki, (k0, kw, _cwa) in enumerate(KSPL):
                    bt = basp.tile([128, 3, 512], F32, name=f"btp{j}_{ki}", tag="bt")
                    nc.gpsimd.dma_start(
                        out=bt[:kw, :, :n], in_=bas_p[k0:k0 + kw, :, n0:n0 + n]
                    )
                    bts[(j, ki)] = bt
            nc.gpsimd.collective_compute(
                "AllReduce",
                ALU.add,
                replica_groups=[list(range(NCORES))],
                ins=[ar_in.ap().opt()],
                outs=[ar_out.ap().opt()],
            )
            lat = latp.tile([B, LAT], F32)
            nc.sync.dma_start(out=lat, in_=ar_out[:, :])

            # ---------------- Phase 1.5: transpose shape params ----------------
            # DVE 32x32 block transposes: spT[ki][r, b] = lat[b, c0+r].
            spT = []
            for (c0, kw, cwa) in KSPL:
                st = latp.tile([cwa, B], F32, name=f"spT{c0}", tag=f"spT{c0}")
                for pb in range(cwa // 32):
                    for fb in range(B // 32):
                        nc.vector.transpose(
                            out=st[32 * pb:32 * pb + 32, 32 * fb:32 * fb + 32],
                            in_=lat[32 * fb:32 * fb + 32,
                                    c0 + 32 * pb:c0 + 32 * pb + 32],
                        )
                spT.append(st)
            nc.tensor.matmul(
                d64, lhsT=spT[3], rhs=spT[3][:, 0:1],
                start=True, stop=True, skip_group_check=True,
            )

            # ---------------- Phase 2: blendshape + fused face transform ----------
            g = Geo(nc, geop)
            # vmean directly from latent: vm = tmpl_mean + shape_p @ basis_mean
            bm_sb = consts.tile([128, 12], F32)
            nc.sync.dma_start(out=bm_sb, in_=bm_p[:, :])
            with tc.tile_pool(name="vmp", bufs=1, space="PSUM") as vmp:
                pvm = vmp.tile([B, 3], F32)
                for ki, (k0, kw, _cwa) in enumerate(KSPL):
                    nc.tensor.matmul(
                        pvm, lhsT=spT[ki][:kw, :], rhs=bm_sb[:kw, ki * 3:ki * 3 + 3],
                        start=(ki == 0), stop=False,
                    )
                nc.tensor.matmul(
                    pvm, lhsT=ones1, rhs=b_sb[:, LAT + 128:LAT + 131],
                    start=False, stop=True,
                )
                vms = geop.tile([B, 3], F32)
                nc.vector.tensor_copy(out=vms, in_=pvm)

            # face rotation matrix, scaled
            aa_face = lat[:, 545:548]
            Rf = axis_angle_R(nc, g, aa_face, "f_", halfpi[:B, :])
            fs = g.t()  # face_scale = latent[551]+1
            nc.vector.tensor_scalar_add(out=fs, in0=lat[:, 551:552], scalar1=1.0)
            Rs = geop.tile([B, 9], F32)
            nc.vector.tensor_scalar_mul(out=Rs, in0=Rf, scalar1=fs)
            # offsets: off_i = face_t_i - sum_l vms_l*Rs[l,i]
            off = geop.tile([B, 3], F32)
            for i in range(3):
                t = g.mul(vms[:, 0:1], Rs[:, i:i + 1])
                t = g.mac(vms[:, 1:2], Rs[:, 3 + i:4 + i], t)
                t = g.mac(vms[:, 2:3], Rs[:, 6 + i:7 + i], t)
                nc.vector.tensor_tensor(
                    out=off[:, i:i + 1], in0=lat[:, 548 + i:549 + i], in1=t,
                    op=ALU.subtract,
                )

            # blendshape chunks; rotation fused per chunk into rt
            rt = planep.tile([B, 3, V], F32)
            with (
                tc.tile_pool(name="tpl", bufs=2) as tplp,
                tc.tile_pool(name="vstage", bufs=3) as vstp,
                tc.tile_pool(name="bpsum", bufs=3, space="PSUM") as bpsum,
            ):

                prev = []  # vstage read-APs for WAR-absorbing dummies
                for j, (n0, n) in enumerate(VCH):
                    vs = vstp.tile([B, 3, 512], F32)
                    if (j, 0) not in bts:
                        for ki, (k0, kw, _cwa) in enumerate(KSPL):
                            bt = basp.tile([128, 3, 512], F32, name=f"btl{j}_{ki}", tag="bt")
                            nc.gpsimd.dma_start(
                                out=bt[:kw, :, :n], in_=bas_p[k0:k0 + kw, :, n0:n0 + n]
                            )
                            bts[(j, ki)] = bt
                    for p in range(3):
                        gi = j * 3 + p
                        if gi >= 3:
                            pap = prev[gi - 3]
                            nc.tensor.matmul(
                                d1, lhsT=pap, rhs=pap,
                                start=True, stop=True, skip_group_check=True,
                            )
                        pv = bpsum.tile([B, 512], F32)
                        for ki, (k0, kw, _cwa) in enumerate(KSPL):
                            nc.tensor.matmul(
                                pv[:, :n],
                                lhsT=spT[ki][:kw, :],
                                rhs=bts[(j, ki)][:kw, p, :n],
                                start=(ki == 0),
                                stop=False,
                            )
                        tl = tplp.tile([1, 512], F32)
                        nc.sync.dma_start(out=tl[:, :n], in_=tpl_p[p:p + 1, n0:n0 + n])
                        nc.tensor.matmul(
                            pv[:, :n], lhsT=ones1, rhs=tl[:, :n],
                            start=False, stop=True,
                        )
                        nc.scalar.copy(out=vs[:, p, :n], in_=pv[:, :n])
                        prev.append(vs[:, p, 0:1])
                    for i in range(3):
                        nc.vector.tensor_scalar(
                            out=rt[:, i, n0:n0 + n], in0=vs[:, 0, :n],
                            scalar1=Rs[:, i:i + 1], scalar2=off[:, i:i + 1],
                            op0=ALU.mult, op1=ALU.add,
                        )
                        for l in (1, 2):
                            nc.vector.scalar_tensor_tensor(
                                out=rt[:, i, n0:n0 + n], in0=vs[:, l, :n],
                                scalar=Rs[:, 3 * l + i:3 * l + i + 1],
                                in1=rt[:, i, n0:n0 + n],
                                op0=ALU.mult, op1=ALU.add,
                            )
            basp_ctx.__exit__(None, None, None)

            # eye processing: both eyes stacked on 128 partitions
            # (rows 0:64 = left batch, 64:128 = right batch)
            EW = 546
            g2 = Geo(nc, geop, rows=128)
            es = geop.tile([128, 3, EW], F32)
            for i in range(3):
                nc.vector.tensor_copy(out=es[0:B, i, :], in_=rt[:, i, l_lo:l_lo + EW])
            nc.sync.dma_start(out=es[B:128, :, :], in_=rt[:, :, r_lo:r_lo + EW])
            # centers (mean over eye verts), both eyes at once
            cc = geop.tile([128, 3], F32)
            for i in range(3):
                nc.vector.tensor_reduce(
                    out=cc[:, i:i + 1], in_=es[:, i, :], axis=AX.X, op=ALU.add
                )
            c3 = geop.tile([128, 3], F32)
            nc.vector.tensor_scalar_mul(out=c3, in0=cc, scalar1=1.0 / EW)
            # pivot verts (l:4051, r:4597)
            pvt = geop.tile([128, 3, 1], F32)
            for i in range(3):
                nc.vector.tensor_copy(out=pvt[0:B, i, :], in_=rt[:, i, 4051:4052])
            nc.sync.dma_start(out=pvt[B:128, :, :], in_=rt[:, :, 4597:4598])
            # a = normalize(pivot - centre)
            a3 = geop.tile([128, 3], F32)
            for i in range(3):
                nc.vector.tensor_tensor(
                    out=a3[:, i:i + 1], in0=pvt[:, i, 0:1], in1=c3[:, i:i + 1],
                    op=ALU.subtract,
                )
            sqe = geop.tile([128, 3], F32)
            nc.vector.tensor_tensor(out=sqe, in0=a3, in1=a3, op=ALU.mult)
            n2 = g2.t()
            nc.vector.tensor_reduce(out=n2, in_=sqe, axis=AX.X, op=ALU.add)
            nn = g2.t()
            nc.scalar.activation(out=nn, in_=n2, func=ACTF.Sqrt)
            rn = g2.t()
            nc.vector.reciprocal(out=rn, in_=nn)
            nc.vector.tensor_scalar_mul(out=a3, in0=a3, scalar1=rn)
            ax, ay, az = a3[:, 0:1], a3[:, 1:2], a3[:, 2:3]
            # find_gaze_R: b=(0,0,GAZE_DIR); v = a x b = (ay*g, -ax*g, 0)
            vx = g2.t()
            nc.vector.tensor_scalar_mul(out=vx, in0=ay, scalar1=GAZE_DIR)
            vy = g2.t()
            nc.vector.tensor_scalar_mul(out=vy, in0=ax, scalar1=-GAZE_DIR)
            cdot = g2.t()
            nc.vector.tensor_scalar_mul(out=cdot, in0=az, scalar1=GAZE_DIR)
            fden = g2.t()
            nc.vector.tensor_scalar_add(out=fden, in0=cdot, scalar1=1.0 + 1e-8)
            f = g2.t()
            nc.vector.reciprocal(out=f, in_=fden)
            vv = g2.mac(vy, vy, g2.mul(vx, vx))
            fvv = g2.mul(f, vv)
            dd = g2.t()  # 1 - f*vv
            nc.vector.tensor_scalar(
                out=dd, in0=fvv, scalar1=-1.0, scalar2=1.0, op0=ALU.mult, op1=ALU.add
            )
            fxy = g2.mul(g2.mul(vx, vy), f)
            Rl = geop.tile([128, 9], F32)
            nc.vector.tensor_tensor(
                out=Rl[:, 0:1], in0=dd, in1=g2.mul(f, g2.mul(vx, vx)), op=ALU.add
            )
            nc.vector.tensor_tensor(
                out=Rl[:, 4:5], in0=dd, in1=g2.mul(f, g2.mul(vy, vy)), op=ALU.add
            )
            nc.vector.tensor_copy(out=Rl[:, 8:9], in_=dd)
            nc.vector.tensor_copy(out=Rl[:, 1:2], in_=fxy)
            nc.vector.tensor_copy(out=Rl[:, 3:4], in_=fxy)
            nc.vector.tensor_copy(out=Rl[:, 2:3], in_=vy)
            nc.vector.tensor_scalar_mul(out=Rl[:, 5:6], in0=vx, scalar1=-1.0)
            nc.vector.tensor_scalar_mul(out=Rl[:, 6:7], in0=vy, scalar1=-1.0)
            nc.vector.tensor_copy(out=Rl[:, 7:8], in_=vx)
            # eyeball rotation from latent rot2 (az=0), stacked l/r
            aa2 = geop.tile([128, 3], F32)
            nc.vector.memset(aa2, 0.0)
            nc.vector.tensor_copy(out=aa2[0:B, 0:2], in_=lat[:, 552:554])
            nc.sync.dma_start(out=aa2[B:128, 0:2], in_=lat[:, 554:556])
            R2 = axis_angle_R(nc, g2, aa2, "e_", halfpi)
            # gaze = GAZE_DIR * R2[2,:]
            gz = geop.tile([128, 3], F32)
            nc.vector.tensor_scalar_mul(out=gz, in0=R2[:, 6:9], scalar1=GAZE_DIR)
            # M = Rl @ R2
            M = geop.tile([128, 9], F32)
            for l in range(3):
                for i in range(3):
                    t = g2.mul(Rl[:, 3 * l:3 * l + 1], R2[:, i:i + 1])
                    t = g2.mac(R2[:, 3 + i:4 + i], Rl[:, 3 * l + 1:3 * l + 2], t)
                    t = g2.mac(R2[:, 6 + i:7 + i], Rl[:, 3 * l + 2:3 * l + 3], t)
                    nc.vector.tensor_copy(out=M[:, 3 * l + i:3 * l + i + 1], in_=t)
            # offe_i = c_i - sum_l c_l M[l,i]
            offe = geop.tile([128, 3], F32)
            for i in range(3):
                t = g2.mul(c3[:, 0:1], M[:, i:i + 1])
                t = g2.mac(c3[:, 1:2], M[:, 3 + i:4 + i], t)
                t = g2.mac(c3[:, 2:3], M[:, 6 + i:7 + i], t)
                nc.vector.tensor_tensor(
                    out=offe[:, i:i + 1], in0=c3[:, i:i + 1], in1=t, op=ALU.subtract
                )
            # apply to both eye slices
            es2 = geop.tile([128, 3, EW], F32)
            for i in range(3):
                nc.vector.tensor_scalar(
                    out=es2[:, i, :], in0=es[:, 0, :],
                    scalar1=M[:, i:i + 1], scalar2=offe[:, i:i + 1],
                    op0=ALU.mult, op1=ALU.add,
                )
                for l in (1, 2):
                    nc.vector.scalar_tensor_tensor(
                        out=es2[:, i, :], in0=es[:, l, :],
                        scalar=M[:, 3 * l + i:3 * l + i + 1], in1=es2[:, i, :],
                        op0=ALU.mult, op1=ALU.add,
                    )
            for i in range(3):
                nc.vector.tensor_copy(out=rt[:, i, l_lo:l_lo + EW], in_=es2[0:B, i, :])
            nc.sync.dma_start(out=rt[:, :, r_lo:r_lo + EW], in_=es2[B:128, :, :])
            # unpack right-eye centre/gaze down to rows 0:64 for the solve
            rc64 = geop.tile([B, 3], F32)
            nc.sync.dma_start(out=rc64, in_=c3[B:128, :])
            rg64 = geop.tile([B, 3], F32)
            nc.sync.dma_start(out=rg64, in_=gz[B:128, :])
            lc = c3[0:B, :]
            lg = gz[0:B, :]
            rc = rc64
            rg = rg64

            # face centre from landmarks
            fc = geop.tile([B, 3], F32)
            for i in range(3):
                t4 = g.add(rt[:, i, idx4[0]:idx4[0] + 1], rt[:, i, idx4[1]:idx4[1] + 1])
                t4 = g.add(t4, rt[:, i, idx4[2]:idx4[2] + 1])
                t4 = g.add(t4, rt[:, i, idx4[3]:idx4[3] + 1])
                t2 = g.add(rt[:, i, idx2[0]:idx2[0] + 1], rt[:, i, idx2[1]:idx2[1] + 1])
                # fc = t4/4/2 + t2/2/2
                o = g.t()
                nc.vector.tensor_scalar_mul(out=o, in0=t4, scalar1=0.125)
                nc.vector.scalar_tensor_tensor(
                    out=fc[:, i:i + 1], in0=t2, scalar=0.25, in1=o,
                    op0=ALU.mult, op1=ALU.add,
                )

            # gaze intersection (Cramer)
            d = [g.sub(rc[:, i:i + 1], lc[:, i:i + 1]) for i in range(3)]
            c0 = [lg[:, i:i + 1] for i in range(3)]
            c1 = []
            for i in range(3):
                o = g.t()
                nc.vector.tensor_scalar_mul(out=o, in0=rg[:, i:i + 1], scalar1=-1.0)
                c1.append(o)
            # c2 = rg x lg
            c2 = list(g.cross3(rg[:, 0:1], rg[:, 1:2], rg[:, 2:3],
                               lg[:, 0:1], lg[:, 1:2], lg[:, 2:3]))
            # w = c1 x c2 ; det = c0.w ; num0 = d.w
            w = g.cross3(*c1, *c2)
            det = g.dot3(*c0, *w)
            num0 = g.dot3(*d, *w)
            # w2 = d x c2 ; num1 = c0.w2  (det with col1 replaced by d)
            w2 = g.cross3(*d, *c2)
            num1 = g.dot3(*c0, *w2)
            rdet = g.t()
            nc.vector.reciprocal(out=rdet, in_=det)
            sol0 = g.mul(num0, rdet)
            sol1 = g.mul(num1, rdet)
            # gp_l = l_c + sol0*lg ; gp_r = r_c + sol1*rg ; gp_mid
            gpl = geop.tile([B, 3], F32)
            gpr = geop.tile([B, 3], F32)
            gpm = geop.tile([B, 3], F32)
            for i in range(3):
                nc.vector.scalar_tensor_tensor(
                    out=gpl[:, i:i + 1], in0=lg[:, i:i + 1], scalar=sol0,
                    in1=lc[:, i:i + 1], op0=ALU.mult, op1=ALU.add,
                )
                nc.vector.scalar_tensor_tensor(
                    out=gpr[:, i:i + 1], in0=rg[:, i:i + 1], scalar=sol1,
                    in1=rc[:, i:i + 1], op0=ALU.mult, op1=ALU.add,
                )
            nc.vector.tensor_tensor(out=gpm, in0=gpl, in1=gpr, op=ALU.add)
            nc.vector.tensor_scalar_mul(out=gpm, in0=gpm, scalar1=0.5)
            dff = geop.tile([B, 3], F32)
            nc.vector.tensor_tensor(out=dff, in0=gpl, in1=gpr, op=ALU.subtract)
            nc.vector.tensor_tensor(out=dff, in0=dff, in1=dff, op=ALU.mult)
            d2 = g.t()
            nc.vector.tensor_reduce(out=d2, in_=dff, axis=AX.X, op=ALU.add)
            dist = g.t()
            nc.scalar.activation(out=dist, in_=d2, func=ACTF.Sqrt)
            # far points l_c + 1000*lg
            farl = geop.tile([B, 3], F32)
            farr = geop.tile([B, 3], F32)
            for i in range(3):
                nc.vector.scalar_tensor_tensor(
                    out=farl[:, i:i + 1], in0=lg[:, i:i + 1], scalar=1000.0,
                    in1=lc[:, i:i + 1], op0=ALU.mult, op1=ALU.add,
                )
                nc.vector.scalar_tensor_tensor(
                    out=farr[:, i:i + 1], in0=rg[:, i:i + 1], scalar=1000.0,
                    in1=rc[:, i:i + 1], op0=ALU.mult, op1=ALU.add,
                )

            # projection of face verts
            cam = geop.tile([B, 12], F32)
            nc.sync.dma_start(out=cam, in_=cam_p[:, :])
            with tc.tile_pool(name="imgp", bufs=1) as imgp:
                img = imgp.tile([B, 3, VM], F32)
                for i in (2, 0, 1):  # z first (feeds the clamp chain on DVE)
                    eng = nc.vector
                    eng.tensor_scalar(
                        out=img[:, i, :], in0=rt[:, 0, 0:VM],
                        scalar1=cam[:, 4 * i:4 * i + 1], scalar2=cam[:, 4 * i + 3:4 * i + 4],
                        op0=ALU.mult, op1=ALU.add,
                    )
                    for l in (1, 2):
                        eng.scalar_tensor_tensor(
                            out=img[:, i, :], in0=rt[:, l, 0:VM],
                            scalar=cam[:, 4 * i + l:4 * i + l + 1], in1=img[:, i, :],
                            op0=ALU.mult, op1=ALU.add,
                        )
                with tc.tile_pool(name="ztmp", bufs=1) as ztp:
                    az_ = ztp.tile([B, VM], F32)
                    nc.scalar.activation(out=az_, in_=img[:, 2, :], func=ACTF.Abs)
                    nc.vector.tensor_scalar_max(out=az_, in0=az_, scalar1=1e-3)
                    sg = ztp.tile([B, VM], F32)
                    nc.vector.tensor_scalar(
                        out=sg, in0=img[:, 2, :], scalar1=0.0, scalar2=None, op0=ALU.is_ge
                    )
                    nc.vector.tensor_scalar(
                        out=sg, in0=sg, scalar1=2.0, scalar2=1.0,
                        op0=ALU.mult, op1=ALU.subtract,
                    )
                    nc.vector.tensor_tensor(out=sg, in0=sg, in1=az_, op=ALU.mult)
                    nc.vector.reciprocal(out=az_, in_=sg)
                    nc.vector.tensor_tensor(
                        out=img[:, 0, :], in0=img[:, 0, :], in1=az_, op=ALU.mult
                    )
                    nc.vector.tensor_tensor(
                        out=img[:, 1, :], in0=img[:, 1, :], in1=az_, op=ALU.mult
                    )

                # landmark gather + tail assembly
                fl = geop.tile([B, 3, 68], F32)
                def _cp(k, out, in_):
                    e = k % 3
                    if e == 0:
                        nc.vector.tensor_copy(out=out, in_=in_)
                    elif e == 1:
                        nc.scalar.copy(out=out, in_=in_)
                    else:
                        nc.gpsimd.tensor_copy(out=out, in_=in_)

                for j, idx in enumerate(fl_idx):
                    for i in range(3):
                        _cp(j * 3 + i, fl[:, i, j:j + 1], rt[:, i, idx:idx + 1])
                tail = geop.tile([B, 3, 11], F32)
                for i in range(3):
                    pieces = [
                        lc[:, i:i + 1], rc[:, i:i + 1], fc[:, i:i + 1],
                        gpl[:, i:i + 1], gpr[:, i:i + 1], gpm[:, i:i + 1],
                        farl[:, i:i + 1], farr[:, i:i + 1],
                        lg[:, i:i + 1], rg[:, i:i + 1], dist,
                    ]
                    for j, src in enumerate(pieces):
                        _cp(i * 11 + j, tail[:, i, j:j + 1], src)

                # output DMAs
                for i in range(3):
                    nc.sync.dma_start(out=out_p[:, i, 0:VM], in_=rt[:, i, 0:VM])
                    nc.sync.dma_start(out=out_p[:, i, VM:2 * VM], in_=img[:, i, :])
                    nc.sync.dma_start(
                        out=out_p[:, i, 2 * VM:2 * VM + 68], in_=fl[:, i, :]
                    )
                    nc.sync.dma_start(
                        out=out_p[:, i, 2 * VM + 68:NOUT], in_=tail[:, i, :]
                    )
    _legalize_waits(nc)
    return nc


def _prep(inputs):
    x = np.ascontiguousarray(inputs["x"].reshape(B, DIN), dtype=np.float32)
    enc_W = np.asarray(inputs["enc_W"], dtype=np.float32)
    basis_np = np.asarray(inputs["shape_basis"], dtype=np.float32)
    tmpl_np = np.asarray(inputs["v_template"], dtype=np.float32)
    enc_b = np.concatenate([
        np.asarray(inputs["enc_b"], dtype=np.float32).reshape(1, LAT),
        np.full((1, B), 1.0 / NCORES, np.float32),
        np.ones((1, B), np.float32),
        tmpl_np.mean(axis=0).reshape(1, 3),
    ], axis=1)
    bmean_full = basis_np.mean(axis=1)  # [400, 3]
    bmean = np.zeros((128, 12), np.float32)
    for ki, (k0, kw) in enumerate([(0, 128), (128, 128), (256, 128), (384, 16)]):
        bmean[:kw, ki * 3:ki * 3 + 3] = bmean_full[k0:k0 + kw]
    tmpl = np.ascontiguousarray(
        np.asarray(inputs["v_template"], dtype=np.float32).T
    )  # [3, V]
    basis = np.ascontiguousarray(
        np.asarray(inputs["shape_basis"], dtype=np.float32).transpose(0, 2, 1)
    )  # [400, 3, V]
    cam = np.ascontiguousarray(
        np.asarray(inputs["camera_parameters"], dtype=np.float32).reshape(B, 12)
    )
    lm = np.asarray(inputs["landmarks"])
    mlm = np.asarray(inputs["masked_landmarks"])
    fmask = np.asarray(inputs["face_mask"])
    lmask = np.asarray(inputs["left_eyeball_mask"])
    rmask = np.asarray(inputs["right_eyeball_mask"])
    assert np.array_equal(lmask, np.arange(lmask[0], lmask[0] + 546)), "lmask not contiguous"
    assert np.array_equal(rmask, np.arange(rmask[0], rmask[0] + 546)), "rmask not contiguous"
    fl_idx = [int(fmask[i]) for i in mlm]
    idx4 = [int(lm[j]) for j in (19, 22, 25, 28)]
    idx2 = [int(lm[j]) for j in (14, 18)]
    return (x, enc_W, enc_b, bmean, tmpl, basis, cam, fl_idx, idx4, idx2,
            int(lmask[0]), int(rmask[0]))


def _run(inputs, trace=False):
    (x, enc_W, enc_b, bmean, tmpl, basis, cam, fl_idx, idx4, idx2, l_lo, r_lo) = _prep(inputs)
    nc = build_graph(fl_idx, idx4, idx2, l_lo, r_lo)
    in_maps = []
    for c in range(NCORES):
        k0 = c * KSH
        in_maps.append({
            "x_sh": np.ascontiguousarray(x[:, k0:k0 + KSH].T),
            "w_sh": np.ascontiguousarray(enc_W[k0:k0 + KSH, :]),
            "enc_b": enc_b,
            "bmean": bmean,
            "tmpl": tmpl,
            "basis": basis,
            "cam": cam,
        })
    res = run_bass_kernel_spmd(
        nc, in_maps, core_ids=list(range(NCORES)), trace=trace
    )
    out = res.results[0]["out"]  # [B, 3, NOUT]
    return np.ascontiguousarray(out.transpose(0, 2, 1)), res


def kernel(**inputs):
    out, _ = _run(inputs, trace=False)
    return out



# revision 42
# speedup vs baseline: 1.6565x; 1.6565x over previous
"""Trainium2 Bass kernel for nn_Autoencoder_65223373357102 (FLAME-style autoencoder).

Strategy (v2):
  Phase 1 (8-way tensor parallel): encoder GEMM [64,150528]@[150528,411] in bf16,
  sharded along the input-feature axis. Only 411 of the 556 latent columns are
  live (cols 400:545 are dead in the reference); the AllReduce payload is bf16.
  Phase 2 (replicated): blendshape GEMM restricted to the 3500 face verts
  (the eye-vertex rotation never reaches the output; everything derived from
  eye verts is linear in the shape params and precomputed host-side into a
  tiny [401,216] matrix: vmean, eye centres, face centre, 68 landmarks).
  The face transform + pinhole projection stream in bf16 through ACT/DVE/Pool
  with verts folded two-per-partition ([128,512] tiles) for full engine width.
  Output is written bf16 and upcast on the host.
"""
import sys
import types

sys.path.insert(0, "/opt/trn_rl_repo")

import numpy as np
import ml_dtypes


def _ensure_ntff_hook():
    """Provide antenv.axon_hooks + install the ctypes NTFF profile hook so
    run_bass_kernel_spmd(trace=True) can pull a neuron-profile under axon."""
    name = "antenv.axon_hooks"
    if name not in sys.modules:
        mod = types.ModuleType(name)
        mod._HOOK = None

        def set_axon_ntff_profile_hook(hook):
            mod._HOOK = hook

        def get_axon_ntff_profile_hook():
            return mod._HOOK

        mod.set_axon_ntff_profile_hook = set_axon_ntff_profile_hook
        mod.get_axon_ntff_profile_hook = get_axon_ntff_profile_hook
        sys.modules[name] = mod
        try:
            import antenv

            antenv.axon_hooks = mod
        except ImportError:
            pass
    mod = sys.modules[name]
    if mod.get_axon_ntff_profile_hook() is None:
        try:
            from trn_agent_boot.trn_boot import _ntff_profile_via_ctypes

            hook = _ntff_profile_via_ctypes("/opt/axon/libaxon_pjrt.so")
            if hook is not None:
                mod.set_axon_ntff_profile_hook(hook)
        except Exception:
            pass


_ensure_ntff_hook()

from concourse import bass, mybir, tile
from concourse.bass_utils import run_bass_kernel_spmd

F32 = mybir.dt.float32
BF16 = mybir.dt.bfloat16
ALU = mybir.AluOpType
ACTF = mybir.ActivationFunctionType
AX = mybir.AxisListType

B = 64
V = 5023
VM = 3500
LAT = 411  # packed live latent cols: 0:400 shape, 400:411 pose
DIN = 3 * 224 * 224  # 150528
NCORES = 8
KSH = DIN // NCORES  # 18816
KTILES = KSH // 128  # 147
TPC = 7  # k-tiles per DMA chunk -> 21 chunks
NOUT = 2 * VM + 68 + 11  # 7079
GAZE_DIR = -1.0
HALF_PI = 1.5707963267948966
# packed latent pose layout (after col 400)
P_ROT = 0  # face_rot  [400:403]
P_T = 3  # face_t     [403:406]
P_SC = 6  # face_scale [406:407]
P_LR = 7  # l_rot      [407:409]
P_RR = 9  # r_rot      [409:411]

# spT k-tiles; last tile = 33 rows: lat cols 384:416 transposed (rows 400:416
# are pose junk, matched against host-zeroed basis rows) + ones row at the
# 32-aligned row 32 (template fold-in).
KSPL = [(0, 128), (128, 128), (256, 128), (384, 33)]
KR = 417  # device-side basis/G row count
# vert blocks: (col0, half-width); each block covers 2*nh verts folded onto
# 128 partitions (rows 0:64 = first half, 64:128 = second half)
BLKS = [(0, 512), (1024, 512), (2048, 512), (3072, 214)]
NG = 72  # tiny-GEMM cols per plane: vmean, lc, rc, fc, fl[68]


class Geo:
    """Helper for tiny per-batch scalar ops on [rows,1] tiles."""

    _uid = [0]

    def __init__(self, nc, pool, rows=B):
        self.nc = nc
        self.pool = pool
        self.rows = rows

    def t(self, cols=1):
        Geo._uid[0] += 1
        return self.pool.tile([self.rows, cols], F32, name=f"g{Geo._uid[0]}_{cols}")

    def mul(self, a, b):
        o = self.t()
        self.nc.vector.tensor_tensor(out=o, in0=a, in1=b, op=ALU.mult)
        return o

    def add(self, a, b):
        o = self.t()
        self.nc.vector.tensor_tensor(out=o, in0=a, in1=b, op=ALU.add)
        return o

    def sub(self, a, b):
        o = self.t()
        self.nc.vector.tensor_tensor(out=o, in0=a, in1=b, op=ALU.subtract)
        return o

    def mac(self, a, s, acc):
        """(a * s) + acc, s is a [rows,1] AP scalar."""
        o = self.t()
        self.nc.vector.scalar_tensor_tensor(
            out=o, in0=a, scalar=s, in1=acc, op0=ALU.mult, op1=ALU.add
        )
        return o

    def dot3(self, ax, ay, az, bx, by, bz):
        o = self.mul(ax, bx)
        o = self.mac(ay, by, o)
        o = self.mac(az, bz, o)
        return o

    def cross3(self, ax, ay, az, bx, by, bz):
        cx = self.sub(self.mul(ay, bz), self.mul(az, by))
        cy = self.sub(self.mul(az, bx), self.mul(ax, bz))
        cz = self.sub(self.mul(ax, by), self.mul(ay, bx))
        return cx, cy, cz


_ENG_ATTR = {
    "SP": "sync", "Pool": "gpsimd", "PE": "tensor",
    "DVE": "vector", "Activation": "scalar",
}


def _legalize_waits(nc):
    """This walrus accepts only one sync-wait slot per instruction; move extra
    waits onto same-engine NoOps inserted right before the instruction."""
    import concourse.mybir as _mybir

    def make_nop(engine):
        eng = getattr(nc, _ENG_ATTR[engine.name])
        bi = eng.nop(nofuse=True)
        mi = bi.ins
        for bb in nc.main_func.blocks:
            if bb.instructions and bb.instructions[-1].name == mi.name:
                bb.instructions.pop()
                break
        mi.engine = engine
        return mi

    for bb in nc.main_func.blocks:
        snapshot = list(bb.instructions)
        newlist = []
        changed = False
        for inst in snapshot:
            si = inst.sync_info
            waits = list(si.on_wait) if (si and si.on_wait) else []
            if (
                len(waits) > 1
                and not inst.name.startswith("barrier")
                and inst.engine is not None
                and getattr(inst.engine, "name", None) in _ENG_ATTR
            ):
                for w in waits[:-1]:
                    nop = make_nop(inst.engine)
                    nop.sync_info = _mybir.SyncInfo(on_wait=[w], on_update=[])
                    newlist.append(nop)
                inst.sync_info = _mybir.SyncInfo(
                    on_wait=[waits[-1]], on_update=list(si.on_update)
                )
                changed = True
            newlist.append(inst)
        if changed:
            bb.instructions[:] = newlist


def build_graph():
    nc = bass.Bass(target_bir_lowering=False)

    NCH = KTILES // TPC
    # hi/lo bf16 splits: exact bf16 products accumulated in fp32 PSUM give
    # ~2^-18 relative precision (the z projection needs ~1e-5 absolute, far
    # beyond plain bf16; see the z-plane cancellation note below)
    xh_p = nc.declare_dram_parameter("x_hi", [NCH, 128, TPC, B], BF16, isOutput=False)
    xl_p = nc.declare_dram_parameter("x_lo", [NCH, 128, TPC, B], BF16, isOutput=False)
    wh_p = nc.declare_dram_parameter("w_hi", [NCH, 128, TPC, LAT], BF16, isOutput=False)
    wl_p = nc.declare_dram_parameter("w_lo", [NCH, 128, TPC, LAT], BF16, isOutput=False)
    b_p = nc.declare_dram_parameter("enc_b", [1, LAT], F32, isOutput=False)
    g_p = nc.declare_dram_parameter("gmat", [KR, 3 * NG], F32, isOutput=False)
    bh_p = nc.declare_dram_parameter("bas_hi", [KR, 6 * VM // 2], BF16, isOutput=False)
    bl_p = nc.declare_dram_parameter("bas_lo", [KR, 6 * VM // 2], BF16, isOutput=False)
    cam_p = nc.declare_dram_parameter("cam", [B, 12], F32, isOutput=False)
    out_p = nc.declare_dram_parameter("out", [B, 3, NOUT], BF16, isOutput=True)

    ar_in = nc.dram_tensor("ar_in", [B, LAT], F32)
    ar_out = nc.dram_tensor("ar_out", [B, LAT], F32, addr_space="Shared")

    with tile.TileContext(nc) as tc, nc.allow_low_precision(reason="bf16 pipeline"):
        with (
            tc.tile_pool(name="consts", bufs=1) as consts,
            tc.tile_pool(name="latents", bufs=1) as latp,
            tc.tile_pool(name="geo", bufs=1) as geop,
            tc.tile_pool(name="bas", bufs=2) as basp,
            tc.tile_pool(name="dum", bufs=1, space="PSUM") as dum,
        ):
            b_sb = consts.tile([1, LAT], F32)
            nc.sync.dma_start(out=b_sb, in_=b_p[:, :])
            ones8 = consts.tile([1, B], F32)
            nc.vector.memset(ones8, 1.0 / NCORES)
            halfpi = consts.tile([128, 1], F32)
            nc.vector.memset(halfpi, HALF_PI)
            c1000 = consts.tile([B, 1], F32)
            nc.vector.memset(c1000, 1000.0)
            cam128 = consts.tile([128, 12], F32)
            nc.sync.dma_start(out=cam128[0:B, :], in_=cam_p[:, :])
            nc.sync.dma_start(out=cam128[B:128, :], in_=cam_p[:, :])
            # tiny-GEMM matrix tiles
            gts = []
            for ki, (k0, kw) in enumerate(KSPL):
                gt = consts.tile([128, 3 * NG], F32, name=f"gt{ki}")
                nc.sync.dma_start(out=gt[:kw, :], in_=g_p[k0:k0 + kw, :])
                gts.append(gt)

            d1 = dum.tile([1, 1], F32)

            # ---------- Phase 1: encoder GEMM (hi/lo bf16, 3 passes) ----------
            with (
                tc.tile_pool(name="xin", bufs=3) as xin,
                tc.tile_pool(name="wts", bufs=3) as wts,
                tc.tile_pool(name="encp", bufs=1, space="PSUM") as encp,
            ):
                pe = encp.tile([B, LAT], F32, name="pe", tag="pe")
                for ci in range(KTILES // TPC):
                    xh_c = xin.tile([128, TPC, B], BF16, tag="xh")
                    nc.gpsimd.dma_start(out=xh_c, in_=xh_p.ap()[ci])
                    xl_c = xin.tile([128, TPC, B], BF16, tag="xl")
                    nc.gpsimd.dma_start(out=xl_c, in_=xl_p.ap()[ci])
                    nc.tensor.matmul(
                        d1, lhsT=xh_c[:, 0, 0:1], rhs=xh_c[:, 0, 0:1],
                        start=True, stop=True, skip_group_check=True,
                    )
                    nc.tensor.matmul(
                        d1, lhsT=xl_c[:, 0, 0:1], rhs=xl_c[:, 0, 0:1],
                        start=True, stop=True, skip_group_check=True,
                    )
                    wh_c = wts.tile([128, TPC, LAT], BF16, tag="wh")
                    nc.sync.dma_start(out=wh_c, in_=wh_p.ap()[ci])
                    wl_c = wts.tile([128, TPC, LAT], BF16, tag="wl")
                    nc.scalar.dma_start(out=wl_c, in_=wl_p.ap()[ci])
                    for t in range(TPC):
                        k = ci * TPC + t
                        # xh stationary shared by the Wh and Wl passes
                        nc.tensor.matmul(
                            pe, lhsT=xh_c[:, t, :], rhs=wh_c[:, t, :],
                            start=(k == 0), stop=False,
                        )
                        nc.tensor.matmul(
                            pe, lhsT=xh_c[:, t, :], rhs=wl_c[:, t, :],
                            start=False, stop=False,
                        )
                        nc.tensor.matmul(
                            pe, lhsT=xl_c[:, t, :], rhs=wh_c[:, t, :],
                            start=False, stop=False,
                        )
                nc.tensor.matmul(pe, lhsT=ones8, rhs=b_sb, start=False, stop=True)
                lat1 = latp.tile([B, LAT], F32)
                nc.vector.tensor_copy(out=lat1, in_=pe)
                nc.sync.dma_start(out=ar_in[:, :], in_=lat1)

            # basis hi/lo tiles: blocks 0/1 prefetched on gpsimd before the
            # collective (fills the AllReduce bubble); blocks 2/3 issued on
            # sync inside the block loop (bufs=2 rotation per tag)
            BOFF = []
            obi = 0
            for c0, nh in BLKS:
                BOFF.append(obi)
                obi += 6 * nh
            bts = {}

            def load_basis_block(bi, eng):
                c0, nh = BLKS[bi]
                for ki, (k0, kw) in enumerate(KSPL):
                    for nm, par in (("h", bh_p), ("l", bl_p)):
                        bt = basp.tile(
                            [128, 3, 1024], BF16,
                            name=f"b{nm}{bi}_{ki}", tag=f"b{nm}{ki}",
                        )
                        eng.dma_start(
                            out=bt[:kw, :, 0:2 * nh],
                            in_=par[k0:k0 + kw, BOFF[bi]:BOFF[bi] + 6 * nh].rearrange(
                                "k (i n) -> k i n", i=3
                            ),
                        )
                        bts[(bi, ki, nm)] = bt

            load_basis_block(0, nc.gpsimd)
            load_basis_block(1, nc.gpsimd)

            nc.gpsimd.collective_compute(
                "AllReduce",
                ALU.add,
                replica_groups=[list(range(NCORES))],
                ins=[ar_in.ap().opt()],
                outs=[ar_out.ap().opt()],
            )
            # fp32 latent, padded to 416 cols: col 411 = 1.0 (second ones row,
            # carries the template third-order residual), cols 412:416 = 0
            lat = latp.tile([B, LAT + 5], F32)
            nc.vector.memset(lat[:, LAT:LAT + 1], 1.0)
            nc.vector.memset(lat[:, LAT + 1:LAT + 5], 0.0)
            nc.sync.dma_start(out=lat[:, 0:LAT], in_=ar_out[:, :])

            # ---------------- transposes: spT k-tiles (fp32 + hi/lo split) ---
            spT = []   # fp32 (for the fp32 tiny-GEMM)
            sph = []   # bf16 hi
            spl = []   # bf16 lo
            for ki, (k0, kw) in enumerate(KSPL):
                cwa = 128 if kw > 64 else 64
                st = latp.tile([cwa, B], F32, name=f"spT{ki}", tag=f"spT{ki}")
                nblk = 4 if kw > 64 else 1
                for pb in range(nblk):
                    for fb in range(B // 32):
                        nc.vector.transpose(
                            out=st[32 * pb:32 * pb + 32, 32 * fb:32 * fb + 32],
                            in_=lat[32 * fb:32 * fb + 32,
                                    k0 + 32 * pb:k0 + 32 * pb + 32],
                        )
                if ki == 3:
                    # ones row (template hi/lo fold-in) at 32-aligned row 32
                    nc.vector.memset(st[32:64, :], 1.0)
                sh = latp.tile([cwa, B], BF16, name=f"sph{ki}", tag=f"sph{ki}")
                nc.vector.tensor_copy(out=sh, in_=st)
                sl = latp.tile([cwa, B], BF16, name=f"spl{ki}", tag=f"spl{ki}")
                nc.vector.tensor_tensor(out=sl, in0=st, in1=sh, op=ALU.subtract)
                spT.append(st)
                sph.append(sh)
                spl.append(sl)
            # pose params fp32 for geometry

            # ---------------- tiny GEMM: derived verts ----------------
            g = Geo(nc, geop)
            g2 = Geo(nc, geop, rows=128)
            drv = geop.tile([B, 3 * NG], F32)
            with tc.tile_pool(name="gps", bufs=1, space="PSUM") as gpsum:
                pg = gpsum.tile([B, 3 * NG], F32)
                nc.tensor.matmul(
                    d1, lhsT=gts[0][:, 0:1], rhs=gts[0][:, 0:1],
                    start=True, stop=True, skip_group_check=True,
                )
                for ki, (k0, kw) in enumerate(KSPL):
                    nc.tensor.matmul(
                        pg, lhsT=spT[ki][:kw, :], rhs=gts[ki][:kw, :],
                        start=(ki == 0), stop=(ki == 3),
                    )
                nc.vector.tensor_copy(out=drv, in_=pg)

            # ---------------- geometry ----------------
            # face rotation: Rf = axis_angle(face_rot); R2 = axis_angle(aa2) for
            # both eyes stacked on 128 partitions.  ACT Sqrt ops batched, then
            # ACT Sin ops batched (table loads).
            aa_f = lat[:, 400 + P_ROT:400 + P_ROT + 3]
            aa2 = geop.tile([128, 3], F32)
            nc.vector.memset(aa2, 0.0)
            nc.vector.tensor_copy(out=aa2[0:B, 0:2], in_=lat[:, 400 + P_LR:400 + P_LR + 2])
            nc.sync.dma_start(out=aa2[B:128, 0:2], in_=lat[:, 400 + P_RR:400 + P_RR + 2])

            def _theta(gg, aa, pfx):
                pool = gg.pool
                rows = gg.rows
                sq = pool.tile([rows, 3], F32, name=pfx + "sq")
                nc.vector.tensor_tensor(out=sq, in0=aa, in1=aa, op=ALU.mult)
                th2 = gg.t()
                nc.vector.tensor_reduce(out=th2, in_=sq, axis=AX.X, op=ALU.add)
                return th2

            th2_f = _theta(g, aa_f, "f_")
            th2_e = _theta(g2, aa2, "e_")
            theta_f = g.t()
            nc.scalar.activation(out=theta_f, in_=th2_f, func=ACTF.Sqrt)
            theta_e = g2.t()
            nc.scalar.activation(out=theta_e, in_=th2_e, func=ACTF.Sqrt)
            s_f = g.t()
            nc.scalar.activation(out=s_f, in_=theta_f, func=ACTF.Sin)
            c_f = g.t()
            nc.scalar.activation(out=c_f, in_=theta_f, func=ACTF.Sin, bias=halfpi[0:B, :])
            s_e = g2.t()
            nc.scalar.activation(out=s_e, in_=theta_e, func=ACTF.Sin)
            c_e = g2.t()
            nc.scalar.activation(out=c_e, in_=theta_e, func=ACTF.Sin, bias=halfpi)

            def _assemble_R(gg, aa, theta, s, c, pfx, Rout=None):
                pool = gg.pool
                rows = gg.rows
                thm = gg.t()
                nc.vector.tensor_scalar_max(out=thm, in0=theta, scalar1=1e-8)
                rth = gg.t()
                nc.vector.reciprocal(out=rth, in_=thm)
                axis3 = pool.tile([rows, 3], F32, name=pfx + "axis")
                nc.vector.tensor_scalar_mul(out=axis3, in0=aa, scalar1=rth)
                omc = gg.t()
                nc.vector.tensor_scalar(
                    out=omc, in0=c, scalar1=-1.0, scalar2=1.0, op0=ALU.mult, op1=ALU.add
                )
                ax, ay, az = axis3[:, 0:1], axis3[:, 1:2], axis3[:, 2:3]
                asq = pool.tile([rows, 3], F32, name=pfx + "asq")
                nc.vector.tensor_tensor(out=asq, in0=axis3, in1=axis3, op=ALU.mult)
                R = Rout if Rout is not None else pool.tile([rows, 9], F32, name=pfx + "R")
                dmul = pool.tile([rows, 3], F32, name=pfx + "dmul")
                nc.vector.tensor_scalar_mul(out=dmul, in0=asq, scalar1=omc)
                sa = pool.tile([rows, 3], F32, name=pfx + "sa")
                nc.vector.tensor_scalar_mul(out=sa, in0=axis3, scalar1=s)
                sax, say, saz = sa[:, 0:1], sa[:, 1:2], sa[:, 2:3]
                mxy = gg.mul(gg.mul(ax, ay), omc)
                mxz = gg.mul(gg.mul(ax, az), omc)
                myz = gg.mul(gg.mul(ay, az), omc)
                for l in range(3):
                    nc.vector.tensor_tensor(
                        out=R[:, 4 * l:4 * l + 1], in0=dmul[:, l:l + 1], in1=c, op=ALU.add
                    )
                nc.vector.tensor_tensor(out=R[:, 1:2], in0=mxy, in1=saz, op=ALU.subtract)
                nc.vector.tensor_tensor(out=R[:, 2:3], in0=mxz, in1=say, op=ALU.add)
                nc.vector.tensor_tensor(out=R[:, 3:4], in0=mxy, in1=saz, op=ALU.add)
                nc.vector.tensor_tensor(out=R[:, 5:6], in0=myz, in1=sax, op=ALU.subtract)
                nc.vector.tensor_tensor(out=R[:, 6:7], in0=mxz, in1=say, op=ALU.subtract)
                nc.vector.tensor_tensor(out=R[:, 7:8], in0=myz, in1=sax, op=ALU.add)
                return R

            Rf = _assemble_R(g, aa_f, theta_f, s_f, c_f, "f_")
            R2 = _assemble_R(g2, aa2, theta_e, s_e, c_e, "e_")

            # rso128: Rs (scaled face rot) cols 0:9, off cols 9:12; rows 64:128
            # duplicated for the folded streaming tiles.
            rso = geop.tile([128, 12], F32)
            fs = g.t()
            nc.vector.tensor_scalar_add(out=fs, in0=lat[:, 400 + P_SC:400 + P_SC + 1], scalar1=1.0)
            nc.vector.tensor_scalar_mul(out=rso[0:B, 0:9], in0=Rf, scalar1=fs)
            Rs = rso[0:B, 0:9]
            for i in range(3):
                t = g.mul(drv[:, 0:1], Rs[:, i:i + 1])
                t = g.mac(drv[:, NG:NG + 1], Rs[:, 3 + i:4 + i], t)
                t = g.mac(drv[:, 2 * NG:2 * NG + 1], Rs[:, 6 + i:7 + i], t)
                nc.vector.tensor_tensor(
                    out=rso[0:B, 9 + i:10 + i], in0=lat[:, 400 + P_T + i:400 + P_T + i + 1],
                    in1=t, op=ALU.subtract,
                )
            off = rso[0:B, 9:12]
            nc.sync.dma_start(out=rso[B:128, :], in_=rso[0:B, :])

            # folded projection: img_i = sum_m Q[i,m]*vs_m + q_i with
            # Q[i,m] = sum_l cam[i,l]*Rs[m,l], q_i = sum_l cam[i,l]*off_l + cam[i,3]
            # qq cols: Q[i,m] at 3m+i, q_i at 9+i (mirrors rso layout)
            qq = geop.tile([128, 12], F32)
            cm = cam128
            for i in range(3):
                for m in range(3):
                    t = g.mul(cm[0:B, 4 * i:4 * i + 1], Rs[:, 3 * m:3 * m + 1])
                    t = g.mac(cm[0:B, 4 * i + 1:4 * i + 2], Rs[:, 3 * m + 1:3 * m + 2], t)
                    t = g.mac(cm[0:B, 4 * i + 2:4 * i + 3], Rs[:, 3 * m + 2:3 * m + 3], t)
                    nc.vector.tensor_copy(out=qq[0:B, 3 * m + i:3 * m + i + 1], in_=t)
                t = g.mul(cm[0:B, 4 * i:4 * i + 1], off[:, 0:1])
                t = g.mac(cm[0:B, 4 * i + 1:4 * i + 2], off[:, 1:2], t)
                t = g.mac(cm[0:B, 4 * i + 2:4 * i + 3], off[:, 2:3], t)
                nc.vector.tensor_tensor(
                    out=qq[0:B, 9 + i:10 + i], in0=t,
                    in1=cm[0:B, 4 * i + 3:4 * i + 4], op=ALU.add,
                )
            nc.sync.dma_start(out=qq[B:128, :], in_=qq[0:B, :])

            # transformed derived verts: tr cols 0=lc 1=rc 2=fc 3:71=fl
            tr = geop.tile([B, 3, NG - 1], F32)
            for i in range(3):
                nc.vector.tensor_scalar(
                    out=tr[:, i, :], in0=drv[:, 1:NG],
                    scalar1=Rs[:, i:i + 1], scalar2=off[:, i:i + 1],
                    op0=ALU.mult, op1=ALU.add,
                )
                for l in (1, 2):
                    nc.vector.scalar_tensor_tensor(
                        out=tr[:, i, :], in0=drv[:, l * NG + 1:l * NG + NG],
                        scalar=Rs[:, 3 * l + i:3 * l + i + 1], in1=tr[:, i, :],
                        op0=ALU.mult, op1=ALU.add,
                    )

            # gazes
            gz = geop.tile([128, 3], F32)
            nc.vector.tensor_scalar_mul(out=gz, in0=R2[:, 6:9], scalar1=GAZE_DIR)
            rg = geop.tile([B, 3], F32)
            nc.sync.dma_start(out=rg, in_=gz[B:128, :])
            lg = gz[0:B, :]

            # tail buffer (fp32): cols 0 lc, 1 rc, 2 fc, 3 gpl, 4 gpr, 5 gpm,
            # 6 farl, 7 farr, 8 lg, 9 rg, 10 dist
            tailf = geop.tile([B, 3, 11], F32)
            for i in range(3):
                nc.vector.tensor_copy(out=tailf[:, i, 0:3], in_=tr[:, i, 0:3])
                nc.vector.tensor_copy(out=tailf[:, i, 8:9], in_=lg[:, i:i + 1])
                nc.vector.tensor_copy(out=tailf[:, i, 9:10], in_=rg[:, i:i + 1])
            lc_c = [tr[:, i, 0:1] for i in range(3)]
            rc_c = [tr[:, i, 1:2] for i in range(3)]
            lg_c = [lg[:, i:i + 1] for i in range(3)]
            rg_c = [rg[:, i:i + 1] for i in range(3)]
            d_c = [g.sub(rc_c[i], lc_c[i]) for i in range(3)]
            c2 = list(g.cross3(*rg_c, *lg_c))  # rg x lg
            w = g.cross3(*c2, *rg_c)  # (-rg) x c2 == c2 x rg
            det = g.dot3(*lg_c, *w)
            num0 = g.dot3(*d_c, *w)
            w2 = g.cross3(*d_c, *c2)
            num1 = g.dot3(*lg_c, *w2)
            rdet = g.t()
            nc.vector.reciprocal(out=rdet, in_=det)
            sol0 = g.mul(num0, rdet)
            sol1 = g.mul(num1, rdet)
            for i in range(3):
                nc.vector.scalar_tensor_tensor(
                    out=tailf[:, i, 3:4], in0=lg_c[i], scalar=sol0,
                    in1=lc_c[i], op0=ALU.mult, op1=ALU.add,
                )
                nc.vector.scalar_tensor_tensor(
                    out=tailf[:, i, 4:5], in0=rg_c[i], scalar=sol1,
                    in1=rc_c[i], op0=ALU.mult, op1=ALU.add,
                )
                gpsum_i = g.add(tailf[:, i, 3:4], tailf[:, i, 4:5])
                nc.vector.tensor_scalar_mul(
                    out=tailf[:, i, 5:6], in0=gpsum_i, scalar1=0.5
                )
                nc.vector.scalar_tensor_tensor(
                    out=tailf[:, i, 6:7], in0=lg_c[i],
                    scalar=c1000, in1=lc_c[i],
                    op0=ALU.mult, op1=ALU.add,
                )
                nc.vector.scalar_tensor_tensor(
                    out=tailf[:, i, 7:8], in0=rg_c[i],
                    scalar=c1000, in1=rc_c[i],
                    op0=ALU.mult, op1=ALU.add,
                )
            # dist^2 (sqrt deferred past the blocks to keep ACT tables stable)
            dffs = [g.sub(tailf[:, i, 3:4], tailf[:, i, 4:5]) for i in range(3)]
            d2 = g.mul(dffs[0], dffs[0])
            d2 = g.mac(dffs[1], dffs[1], d2)
            d2 = g.mac(dffs[2], dffs[2], d2)

            # ---------------- streaming blocks ----------------
            with (
                tc.tile_pool(name="rtp", bufs=2) as rtp,
                tc.tile_pool(name="bps", bufs=2, space="PSUM") as bps,
            ):
                for bi, (c0, nh) in enumerate(BLKS):
                    if bi + 2 < len(BLKS):
                        load_basis_block(bi + 2, nc.sync)
                    pls = [
                        bps.tile([128, 512], F32, name=f"pl{bi}_{i}", tag=f"pl{i}")
                        for i in range(3)
                    ]
                    for ki, (k0, kw) in enumerate(KSPL):
                        bh = bts[(bi, ki, "h")]
                        bl = bts[(bi, ki, "l")]
                        nc.tensor.matmul(
                            d1, lhsT=bh[0:1, 0, 0:1], rhs=bh[0:1, 0, 0:1],
                            start=True, stop=True, skip_group_check=True,
                        )
                        nc.tensor.matmul(
                            d1, lhsT=bl[0:1, 0, 0:1], rhs=bl[0:1, 0, 0:1],
                            start=True, stop=True, skip_group_check=True,
                        )
                        # 3 hi/lo passes; sph stationary shared by passes 1+2
                        for pi, (lhs, bt) in enumerate(
                            ((sph[ki], bh), (sph[ki], bl), (spl[ki], bh))
                        ):
                            st0 = ki == 0 and pi == 0
                            sp1 = ki == 3 and pi == 2
                            for i in range(3):
                                nc.tensor.matmul(
                                    pls[i][0:B, 0:nh], lhsT=lhs[:kw, :],
                                    rhs=bt[:kw, i, 0:nh],
                                    start=st0, stop=sp1,
                                )
                                nc.tensor.matmul(
                                    pls[i][B:128, 0:nh], lhsT=lhs[:kw, :],
                                    rhs=bt[:kw, i, nh:2 * nh],
                                    start=st0, stop=sp1,
                                )
                    rtt = rtp.tile([128, 3, 512], BF16, name=f"rt{bi}", tag="rt")
                    imgt = rtp.tile([128, 3, 512], BF16, name=f"img{bi}", tag="img")
                    rzt = rtp.tile([128, 512], BF16, name=f"rz{bi}", tag="rz")
                    zft = rtp.tile([128, 512], F32, name=f"zf{bi}", tag="zf")
                    rt = rtt[:, :, 0:nh]
                    img = imgt[:, :, 0:nh]
                    rz = rzt[:, 0:nh]
                    zf = zft[:, 0:nh]
                    # rt_i and img_i are independent 3-term mul-add chains
                    # straight from PSUM (img via the cam@Rs folded transform).
                    # The z plane stays fp32: z ~ 0 comes from cancelling O(5)
                    # terms, and 1/z flips sign under bf16 rounding.
                    chains = [(qq, zf, 2), (rso, rt, 0), (rso, rt, 1),
                              (rso, rt, 2), (qq, img, 0), (qq, img, 1)]
                    for S, dst_t, i in chains:
                        dst = dst_t if dst_t is zf else dst_t[:, i, :]
                        nc.scalar.activation(
                            out=dst, in_=pls[0][:, 0:nh],
                            func=ACTF.Identity,
                            scale=S[:, i:i + 1], bias=S[:, 9 + i:10 + i],
                        )
                        nc.vector.scalar_tensor_tensor(
                            out=dst, in0=pls[1][:, 0:nh],
                            scalar=S[:, 3 + i:4 + i],
                            in1=dst, op0=ALU.mult, op1=ALU.add,
                        )
                        nc.vector.scalar_tensor_tensor(
                            out=dst, in0=pls[2][:, 0:nh],
                            scalar=S[:, 6 + i:7 + i],
                            in1=dst, op0=ALU.mult, op1=ALU.add,
                        )
                    nc.gpsimd.tensor_copy(out=img[:, 2, :], in_=zf)
                    nc.vector.reciprocal(out=rz, in_=zf)
                    nc.vector.tensor_scalar(
                        out=rz, in0=rz, scalar1=-1000.0, scalar2=1000.0,
                        op0=ALU.max, op1=ALU.min,
                    )
                    nc.vector.tensor_tensor(
                        out=img[:, 0, :], in0=img[:, 0, :], in1=rz, op=ALU.mult
                    )
                    nc.gpsimd.tensor_tensor(
                        out=img[:, 1, :], in0=img[:, 1, :], in1=rz, op=ALU.mult
                    )
                    # output DMAs (two halves per tile)
                    nc.sync.dma_start(out=out_p[:, :, c0:c0 + nh], in_=rt[0:B, :, :])
                    nc.sync.dma_start(
                        out=out_p[:, :, c0 + nh:c0 + 2 * nh], in_=rt[B:128, :, :]
                    )
                    nc.sync.dma_start(
                        out=out_p[:, :, VM + c0:VM + c0 + nh], in_=img[0:B, :, :]
                    )
                    nc.sync.dma_start(
                        out=out_p[:, :, VM + c0 + nh:VM + c0 + 2 * nh],
                        in_=img[B:128, :, :],
                    )

            # ---------------- tail: dist sqrt + assembly ----------------
            dist = g.t()
            nc.scalar.activation(out=dist, in_=d2, func=ACTF.Sqrt)
            for i in range(3):
                nc.vector.tensor_copy(out=tailf[:, i, 10:11], in_=dist)
            tailb = geop.tile([B, 3, 68 + 11], BF16)
            nc.vector.tensor_copy(out=tailb[:, :, 0:68], in_=tr[:, :, 3:NG - 1])
            nc.vector.tensor_copy(out=tailb[:, :, 68:79], in_=tailf)
            nc.sync.dma_start(out=out_p[:, :, 2 * VM:NOUT], in_=tailb)

    _legalize_waits(nc)
    return nc


def _hilo(a):
    bf16 = ml_dtypes.bfloat16
    hi = a.astype(bf16)
    lo = (a - hi.astype(np.float32)).astype(bf16)
    return hi, lo


def _prep(inputs):
    bf16 = ml_dtypes.bfloat16
    x = np.asarray(inputs["x"], dtype=np.float32).reshape(B, DIN)
    xT = np.ascontiguousarray(x.T)  # [DIN, B] fp32
    enc_W = np.asarray(inputs["enc_W"], dtype=np.float32)
    w_pack = np.ascontiguousarray(
        np.concatenate([enc_W[:, 0:400], enc_W[:, 545:556]], axis=1)
    )  # [DIN, 411] fp32
    enc_b = np.asarray(inputs["enc_b"], dtype=np.float32)
    b_pack = np.ascontiguousarray(
        np.concatenate([enc_b[0:400], enc_b[545:556]]).reshape(1, LAT)
    )

    tmpl = np.asarray(inputs["v_template"], dtype=np.float32)  # [V, 3]
    basis = np.asarray(inputs["shape_basis"], dtype=np.float32)  # [400, V, 3]
    # device rows: 0:400 = basis, 400:416 = zeros (junk-latent guard; row 411
    # pairs with the lat-pad ones column and carries the template 3rd-order
    # residual in the hi stream), 416 = template
    big = np.concatenate(
        [basis, np.zeros((16, V, 3), np.float32), tmpl[None]], axis=0
    )  # [KR, V, 3]
    bigT = np.ascontiguousarray(big[:, 0:VM, :].transpose(0, 2, 1))  # [KR, 3, VM] f32
    bT_h, bT_l = _hilo(bigT)
    # template third-order residual -> hi stream row 411 (lo row 411 stays 0)
    tplT = bigT[KR - 1]  # [3, VM]
    tpl_ll = tplT - bT_h[KR - 1].astype(np.float32) - bT_l[KR - 1].astype(np.float32)
    bT_h[411] = tpl_ll.astype(bf16)
    # block-major packing for contiguous per-partition DMA rows
    def pack_blocks(m):
        segs = []
        for c0, nh in BLKS:
            segs.append(m[:, :, c0:c0 + 2 * nh].reshape(KR, 6 * nh))
        return np.ascontiguousarray(np.concatenate(segs, axis=1))
    bas_h = pack_blocks(bT_h)
    bas_l = pack_blocks(bT_l)

    lm = np.asarray(inputs["landmarks"])
    mlm = np.asarray(inputs["masked_landmarks"])
    fmask = np.asarray(inputs["face_mask"])
    lmask = np.asarray(inputs["left_eyeball_mask"])
    rmask = np.asarray(inputs["right_eyeball_mask"])
    assert np.array_equal(fmask, np.arange(VM)), "face_mask not arange"
    fl_idx = np.asarray([int(fmask[i]) for i in mlm])
    idx4 = np.asarray([int(lm[j]) for j in (19, 22, 25, 28)])
    idx2 = np.asarray([int(lm[j]) for j in (14, 18)])

    cols = [
        big.mean(axis=1),                       # vmean (raw, pre-transform)
        big[:, lmask, :].mean(axis=1),          # l_c
        big[:, rmask, :].mean(axis=1),          # r_c
        big[:, idx4, :].mean(axis=1) / 2.0 + big[:, idx2, :].mean(axis=1) / 2.0,
    ] + [big[:, j, :] for j in fl_idx]           # 68 landmarks
    G = np.stack(cols, axis=1)  # [KR, NG, 3]
    G = np.ascontiguousarray(G.transpose(0, 2, 1).reshape(KR, 3 * NG))  # fp32

    cam = np.ascontiguousarray(
        np.asarray(inputs["camera_parameters"], dtype=np.float32).reshape(B, 12)
    )
    return xT, w_pack, b_pack, bas_h, bas_l, G, cam


def _run(inputs, trace=False):
    xT, w_pack, b_pack, bas_h, bas_l, G, cam = _prep(inputs)
    nc = build_graph()
    NCH = KTILES // TPC

    def chunked(m, ncols):
        return np.ascontiguousarray(
            m.reshape(NCH, TPC, 128, ncols).transpose(0, 2, 1, 3)
        )

    in_maps = []
    for c in range(NCORES):
        k0 = c * KSH
        xh, xl = _hilo(xT[k0:k0 + KSH, :])
        wh, wl = _hilo(w_pack[k0:k0 + KSH, :])
        in_maps.append({
            "x_hi": chunked(xh, B),
            "x_lo": chunked(xl, B),
            "w_hi": chunked(wh, LAT),
            "w_lo": chunked(wl, LAT),
            "enc_b": b_pack,
            "gmat": G,
            "bas_hi": bas_h,
            "bas_lo": bas_l,
            "cam": cam,
        })
    res = run_bass_kernel_spmd(
        nc, in_maps, core_ids=list(range(NCORES)), trace=trace
    )
    out = np.asarray(res.results[0]["out"], dtype=np.float32)  # [B, 3, NOUT]
    return np.ascontiguousarray(out.transpose(0, 2, 1)), res


def kernel(**inputs):
    out, _ = _run(inputs, trace=False)
    return out
